# revision 13
# baseline (speedup 1.0000x reference)
"""MoE grouped-MLP (Megatron GroupedMLP fwd, no gate) on 8 TRN2 NeuronCores.

Strategy (F8 tensor-parallel): every core processes ALL 8192 tokens; the
FFN dimension F=4096 is split 8 ways (fh=512 per core), for every expert.
Each core holds all 8 experts' [H, fh] / [fh, H] weight slices resident in
SBUF (16.8 MB bf16) and walks the tokens in expert-pure ragged tiles
(full 512-wide tiles plus one remainder tile per expert), so the token
layout — and therefore the traced program — is identical on every core.
This gives perfect load balance (1024 token-equivalents per core, the
8-core floor) with no padding.

Each core emits a partial fc2 output (its fh-slice of the F contraction)
in fp32; the host sums the 8 partials.  All matmuls run transposed
(fc1^T = w1^T @ x^T, out^T = w2^T @ act^T) so both weight operands load
in their natural [K, M] layouts and no on-device transposes are needed.

DMA traffic is consolidated into one instruction per logical transfer
(x tile / out tile / expert weight set) via 3-D access patterns, spread
over three issue paths: weights on the SP HWDGE ring, x loads on the
ACT HWDGE ring, out stores on gpsimd SWDGE — so x prefetch never queues
behind the 16.8 MB weight preload and the Q7 descriptor generator is
nowhere near saturation.
"""

from contextlib import ExitStack

import ml_dtypes
import numpy as np

import concourse.bass as bass
import concourse.mybir as mybir
import concourse.tile as tile
from concourse import bacc
from concourse.bass_utils import run_bass_kernel_spmd

NTILE = 512  # max token tile (moving-operand free dim; one fp32 PSUM bank)
BF16 = mybir.dt.bfloat16
F32 = mybir.dt.float32
NP_BF16 = ml_dtypes.bfloat16

E, H, F, T = 8, 1024, 4096, 8192
FH = F // 8  # per-core ffn slice
OUT_DT = BF16  # partial-output dtype (host sums 8 partials in fp32)

_NC_CACHE = {}


def _build(tiles, h, fh):
    """Trace the SPMD bass program: one core's fh-slice of the full MLP.

    tiles: tuple of (expert, token_offset, width) — identical on all cores.
    """
    key = (tiles, h, fh)
    if key in _NC_CACHE:
        return _NC_CACHE[key]

    n_exp = E
    p_tok = T
    kh = h // 128    # fc1 contraction tiles
    kf = fh // 128   # fc2 contraction tiles
    m1 = fh // 128   # fc1 output partition tiles
    m2 = h // 128    # fc2 output partition tiles

    nc = bacc.Bacc()
    xT = nc.dram_tensor("xT", [h, p_tok], BF16, kind="ExternalInput")
    w1h = nc.dram_tensor("w1h", [n_exp, h, fh], BF16, kind="ExternalInput")
    w2h = nc.dram_tensor("w2h", [n_exp, fh, h], BF16, kind="ExternalInput")
    outT = nc.dram_tensor("outT", [h, p_tok], OUT_DT, kind="ExternalOutput")

    # k-major partition views: row (128k + p) -> [p, k, cols]
    xT_v = xT.rearrange("(k p) t -> p k t", p=128)
    outT_v = outT.rearrange("(m p) t -> p m t", p=128)

    with tile.TileContext(nc) as tc, ExitStack() as ctx:
        wpool = ctx.enter_context(tc.tile_pool(name="weights", bufs=1))
        xpool = ctx.enter_context(tc.tile_pool(name="x", bufs=4))
        apool = ctx.enter_context(tc.tile_pool(name="act", bufs=2))
        opool = ctx.enter_context(tc.tile_pool(name="out", bufs=3))
        ps1 = ctx.enter_context(tc.tile_pool(name="ps1", bufs=4, space="PSUM"))
        ps2 = ctx.enter_context(tc.tile_pool(name="ps2", bufs=4, space="PSUM"))

        # Resident weights: one DMA per (expert, matrix), expert 0 first so
        # the first tile's compute starts as soon as its set lands.
        w1_sb, w2_sb = {}, {}
        kh2 = kh // 2
        for e in range(n_exp):
            # w1 split in two halves of the contraction so the first tile's
            # matmuls start after 0.5 MB instead of 1 MB of weight DMA.
            w1v = w1h[e].rearrange("(k p) f -> p k f", p=128)
            for half in range(2):
                t1 = wpool.tile([128, kh2, fh], BF16, name=f"w1_{e}_{half}")
                nc.sync.dma_start(
                    out=t1, in_=w1v[:, half * kh2 : (half + 1) * kh2, :]
                )
                w1_sb[e, half] = t1
            t2 = wpool.tile([128, kf, h], BF16, name=f"w2_{e}")
            nc.sync.dma_start(out=t2, in_=w2h[e].rearrange("(k p) f -> p k f", p=128))
            w2_sb[e] = t2

        for e, off, w in tiles:
            xt = xpool.tile([128, kh, w], BF16, name="x", tag="x")
            nc.scalar.dma_start(out=xt, in_=xT_v[:, :, off : off + w])

            act_n = []
            for m in range(m1):
                ps = ps1.tile([128, w], F32, name="fc1ps", tag="fc1ps")
                for k in range(kh):
                    nc.tensor.matmul(
                        ps,
                        w1_sb[e, k // kh2][:, k % kh2, 128 * m : 128 * (m + 1)],
                        xt[:, k, :],
                        start=(k == 0),
                        stop=(k == kh - 1),
                    )
                a = apool.tile([128, w], BF16, name=f"a_{m}", tag=f"a{m}")
                nc.scalar.activation(a, ps, mybir.ActivationFunctionType.Gelu)
                act_n.append(a)

            ot = opool.tile([128, m2, w], OUT_DT, name="o", tag="o")
            for m in range(m2):
                ps = ps2.tile([128, w], F32, name="fc2ps", tag="fc2ps")
                for k in range(kf):
                    nc.tensor.matmul(
                        ps,
                        w2_sb[e][:, k, 128 * m : 128 * (m + 1)],
                        act_n[k],
                        start=(k == 0),
                        stop=(k == kf - 1),
                    )
                nc.vector.tensor_copy(ot[:, m, :], ps)
            nc.gpsimd.dma_start(out=outT_v[:, :, off : off + w], in_=ot)

    nc.compile()  # bacc legalization: splits multi-wait DMAs for TRN2 codegen
    _NC_CACHE[key] = nc
    return nc


def _plan(tokens_per_expert):
    """Expert-pure ragged tiles over the token axis (identical on all cores)."""
    tpe = np.asarray(tokens_per_expert, dtype=np.int64)
    full, rest = [], []
    off = 0
    for e in range(len(tpe)):
        left = int(tpe[e])
        while left > 0:
            w = min(NTILE, left)
            # 512-wide tiles keep DMA chunks at aligned 1 KiB; remainder
            # tiles (LDWEIGHTS-floor-bound on PE) run last so the final
            # out-DMA + drain tail is tiny.
            (full if w == NTILE else rest).append((e, off, w))
            off += w
            left -= w
    return tpe, tuple(full + sorted(rest, key=lambda t: -t[2]))


def prepare(dispatched_input, tokens_per_expert, w1, w2):
    """Build (nc, in_maps, gather) for the F8 tensor-parallel SPMD program."""
    t_tot, h = dispatched_input.shape
    n_exp, _, f = w1.shape
    fh = f // 8
    tpe, tiles = _plan(tokens_per_expert)

    nc = _build(tiles, h, fh)

    xT = np.ascontiguousarray(dispatched_input.astype(NP_BF16).T)
    w1_bf = w1.astype(NP_BF16)
    w2_bf = w2.astype(NP_BF16)
    in_maps = []
    for c in range(8):
        fs = slice(c * fh, (c + 1) * fh)
        in_maps.append(
            {
                "xT": xT,
                "w1h": np.ascontiguousarray(w1_bf[:, :, fs]),
                "w2h": np.ascontiguousarray(w2_bf[:, fs, :]),
            }
        )

    def gather(per_core_out):
        # Two independent accumulators pipeline the bf16->f32 casts better.
        a = per_core_out[0].astype(np.float32)
        b = per_core_out[1].astype(np.float32)
        for c in range(2, 8, 2):
            a += per_core_out[c].astype(np.float32)
            b += per_core_out[c + 1].astype(np.float32)
        a += b
        return a.T

    return nc, in_maps, gather


def kernel(dispatched_input, tokens_per_expert, w1, w2, _spmd_kwargs=None):
    nc, in_maps, gather = prepare(dispatched_input, tokens_per_expert, w1, w2)
    res = run_bass_kernel_spmd(
        nc, in_maps, core_ids=list(range(8)), **(_spmd_kwargs or {})
    )
    global LAST_RESULT
    LAST_RESULT = res
    return gather([r["outT"] for r in res.results])
